# revision 1
# baseline (speedup 1.0000x reference)
"""LocallyConnected1d (B=32, C=32, L=4096, K=7, stride=1) Trainium2 Bass kernel.

Strategy (hardcoded for this problem):
  - Shard L_out=4090 across 8 cores (sequence parallel), 512 positions/core
    (padded; core 7 carries 6 zero-padded positions). Each weight element is
    read from HBM exactly once.
  - Host pre-permutes the operands into PE-friendly layouts:
      x2 [128, 32*516]: partition (tap-band kk in 0..3, in_C i), col (b, c),
                        value x[b, i, 512m + c + kk]
      w1 [128, 32*512]: partition (kk, i), col (o, l), taps 0..3
      w2 [ 96, 32*512]: partition (kk, i), col (o, l), taps 4..6
  - Per output position l: two accumulating matmuls with contraction over
    (tap, in_C) = 128 resp. 96 partitions:
      psum[b, o] += x2[:, (b, l)]^T . w1[:, (o, l)]     (taps 0-3)
      psum[b, o] += x2[:96, (b, l+4)]^T . w2[:, (o, l)] (taps 4-6)
    Output column group cg = l % 4 -> psum partitions [32cg, 32cg+32), so four
    consecutive positions stream concurrently on the PE's four column groups.
  - PSUM: one 2 KB bank holds 64 positions (4 cgs x 16 slots x 32 out_C);
    bank start/stop via the per-position start=True (lazy zero) / stop=True.
    Banks ping-pong (bufs=2); a finished bank is drained to SBUF by VectorE
    and the whole staged output leaves in one 2 MB DMA (host un-permutes).
"""

import sys

if "/opt/trn_rl_repo" not in sys.path:
    sys.path.insert(0, "/opt/trn_rl_repo")

import numpy as np

import bass_rust
from concourse import bass, mybir, tile
from concourse.bass_utils import run_bass_kernel_spmd

# Problem constants (hardcoded; must match the grading reference).
B = 32          # batch
IC = 32         # in channels
L = 4096        # input length
OC = 32         # out channels
K = 7           # kernel taps
L_OUT = 4090    # (L - (K-1)) // 1

NCORES = 8
LP = 512        # positions per core (padded: 8*512 = 4096 >= 4090)
XE = LP + 4     # x2 column extent (stationary cols l and l+4, taps +0..3)
CHUNK = 128     # weight positions per DMA chunk
NCHUNK = LP // CHUNK

X2COLS = B * XE          # x2 per-partition cols: b*XE + c
WCOLS = OC * LP          # w1/w2 per-partition cols: o*LP + l
WCCOLS = OC * CHUNK      # chunk tile cols: o*CHUNK + l_loc
OCOLS = OC * (LP // 4)   # out-stage per-partition cols: o*128 + t, t = l//4

F32 = mybir.dt.float32

_CACHE = {}


def _ap(t_ap, offset, dims):
    """Build a raw access pattern on the tensor behind an AP."""
    return bass_rust.AP(t_ap.tensor, int(offset), [[int(s), int(n)] for s, n in dims])


def _emit(reps=None):
    """Build the (identical-per-core) single-core program.

    reps: if set, wrap the whole body (DMAs included) in a hardware loop that
    executes it `reps` times -- used only for wall-clock timing calibration.
    """
    import contextlib

    nc = bass.Bass()
    x_d = nc.dram_tensor("x2", [128, X2COLS], F32, kind="ExternalInput")
    w1_d = nc.dram_tensor("w1", [128, WCOLS], F32, kind="ExternalInput")
    w2_d = nc.dram_tensor("w2", [96, WCOLS], F32, kind="ExternalInput")
    o_d = nc.dram_tensor("out", [128, OCOLS], F32, kind="ExternalOutput")

    with tile.TileContext(nc) as tc:
        with (
            tc.tile_pool(name="persist", bufs=1) as persist,
            tc.tile_pool(name="w1pool", bufs=3) as w1pool,
            tc.tile_pool(name="w2pool", bufs=3) as w2pool,
            tc.tile_pool(name="psum", bufs=2, space=bass.MemorySpace.PSUM) as psum,
        ):
            x2 = persist.tile([128, X2COLS], F32, name="x2t")
            x2a = x2[:]
            ost = persist.tile([128, OCOLS], F32, name="ostage")
            osa = ost[:]

            loop = (
                tc.For_i(0, reps, 1, hint_engines=(mybir.EngineType.PE,))
                if reps is not None else contextlib.nullcontext()
            )
            with loop:
                _emit_body(nc, x2a, osa, x_d, w1_d, w2_d, o_d,
                           w1pool, w2pool, psum)
    _split_matmul_waits(nc)
    return nc


def _emit_body(nc, x2a, osa, x_d, w1_d, w2_d, o_d, w1pool, w2pool, psum):
    nc.sync.dma_start(x2a, x_d[:])

    w1c = [None] * NCHUNK
    w2c = [None] * NCHUNK
    pg = None

    for l in range(LP):
        j, l_loc = divmod(l, CHUNK)
        t, cg = divmod(l, 4)
        g, s = divmod(t, 16)  # psum bank index, slot within bank

        if l_loc == 0:
            # host packs weights chunk-major: chunk j = cols [j*WCCOLS, ...),
            # inner (o, l_loc) -> fully contiguous 16 KB/partition DMA rows
            w1t = w1pool.tile([128, WCCOLS], F32, tag="w1c", name=f"w1c{j}")
            w1c[j] = w1t[:]
            nc.sync.dma_start(
                _ap(w1c[j], 0, [[WCCOLS, 128], [1, WCCOLS]]),
                _ap(w1_d[:], j * WCCOLS, [[WCOLS, 128], [1, WCCOLS]]),
            )
            w2t = w2pool.tile([128, WCCOLS], F32, tag="w2c", name=f"w2c{j}")
            w2c[j] = w2t[:]
            nc.sync.dma_start(
                _ap(w2c[j], 0, [[WCCOLS, 96], [1, WCCOLS]]),
                _ap(w2_d[:], j * WCCOLS, [[WCOLS, 96], [1, WCCOLS]]),
            )
        if l % 64 == 0:
            pgt = psum.tile([128, 512], F32, tag="ps", name=f"ps{g}")
            pg = pgt[:]
        out_ap = _ap(pg, 32 * cg * 512 + s * 32, [[512, 32], [1, 32]])
        # taps 0..3: contraction over 128 partitions
        nc.tensor.matmul(
            out_ap,
            _ap(x2a, l, [[X2COLS, 128], [XE, B]]),
            _ap(w1c[j], l_loc, [[WCCOLS, 128], [CHUNK, OC]]),
            start=True, stop=False,
            tile_position=(0, 32 * cg), skip_group_check=True,
        )
        # taps 4..6: contraction over 96 partitions (x shifted by 4)
        nc.tensor.matmul(
            out_ap,
            _ap(x2a, l + 4, [[X2COLS, 96], [XE, B]]),
            _ap(w2c[j], l_loc, [[WCCOLS, 96], [CHUNK, OC]]),
            start=False, stop=True,
            tile_position=(0, 32 * cg), skip_group_check=True,
        )
        if l % 64 == 63:
            # bank holds positions [l-63, l]: drain to OutStage
            nc.vector.tensor_copy(
                _ap(osa, g * 16, [[OCOLS, 128], [1, 16], [128, OC]]),
                _ap(pg, 0, [[512, 128], [32, 16], [1, 32]]),
            )

    nc.sync.dma_start(o_d[:], osa)


def _split_matmul_waits(nc):
    """This walrus build allows at most one sync wait per instruction.
    Relocate each multi-wait instruction's waits onto a chain of single-wait
    NoOps inserted just before it on the same engine -- program order makes
    this semantically identical."""
    for f in nc.m.functions:
        for bb in f.blocks:
            insts = list(bb.instructions)
            out = []
            changed = False
            for ins in insts:
                si = ins.sync_info
                if (si is not None and si.on_wait
                        and len(si.on_wait) >= 2):
                    for w in si.on_wait:
                        nop = mybir.InstNoOp(
                            name=nc.get_next_instruction_name(),
                            ins=[], outs=[],
                            sync_info=mybir.SyncInfo(
                                on_wait=[w], on_update=[]),
                            bass_nofuse=True,
                            engine=ins.engine,
                        )
                        nc.inst_map[nop.name] = nop
                        out.append(nop)
                    ins.sync_info = mybir.SyncInfo(
                        on_wait=[], on_update=list(si.on_update))
                    changed = True
                out.append(ins)
            if changed:
                bb.instructions = out


def _get_nc():
    if "nc" not in _CACHE:
        _CACHE["nc"] = _emit()
    return _CACHE["nc"]


def _shard_inputs(x, weight):
    """Pre-permute full inputs into the per-core kernel layouts."""
    x = np.asarray(x, dtype=np.float32)
    weight = np.asarray(weight, dtype=np.float32)
    xpad = np.zeros((B, IC, NCORES * LP + XE + 4), dtype=np.float32)
    xpad[:, :, :L] = x
    wpad = np.zeros((OC, IC, NCORES * LP, K), dtype=np.float32)
    wpad[:, :, :L_OUT, :] = weight

    in_maps = []
    for m in range(NCORES):
        l0 = m * LP
        win = xpad[:, :, l0 : l0 + XE + 3]  # (B, IC, XE+3)
        x2 = np.empty((4, IC, B, XE), dtype=np.float32)
        for kk in range(4):
            x2[kk] = win[:, :, kk : kk + XE].transpose(1, 0, 2)
        ws = wpad[:, :, l0 : l0 + LP, :]        # (OC, IC, LP, K)
        wt = ws.transpose(3, 1, 0, 2)           # (K, IC, OC, LP)
        # chunk-major columns: (NCHUNK, OC, CHUNK) so each chunk DMA is
        # one fully contiguous 16 KB-per-partition transfer
        wt = wt.reshape(K, IC, OC, NCHUNK, CHUNK).transpose(0, 1, 3, 2, 4)
        in_maps.append({
            "x2": np.ascontiguousarray(x2).reshape(128, X2COLS),
            "w1": np.ascontiguousarray(wt[0:4]).reshape(128, WCOLS),
            "w2": np.ascontiguousarray(wt[4:7]).reshape(96, WCOLS),
        })
    return in_maps


def _unshard_output(res):
    """res: list of per-core {"out": (128, OCOLS)} -> full (B, OC, L_OUT)."""
    out = np.empty((B, OC, NCORES * LP), dtype=np.float32)
    for m in range(NCORES):
        arr = res[m]["out"].reshape(4, B, OC, LP // 4)  # (cg, b, o, t)
        out[:, :, m * LP : (m + 1) * LP] = (
            arr.transpose(1, 2, 3, 0).reshape(B, OC, LP)
        )
    return np.ascontiguousarray(out[:, :, :L_OUT])


def kernel(x, weight):
    nc = _get_nc()
    in_maps = _shard_inputs(x, weight)
    res = run_bass_kernel_spmd(nc, in_maps, list(range(NCORES))).results
    return _unshard_output(res)

